# revision 29
# baseline (speedup 1.0000x reference)
"""CrossAttentionFusion Trainium2 kernel.

Full inputs -> shard (batch x query-half) over 8 NeuronCores -> full output.

Per core (batch b = core//2, query half h = core%2, NH=2048 queries):
  Algebraic folding (host precompute):
    L[m,n] = K^T Q = x2^T (k_w^T q_w) x1 =: x2^T Q'   (K never materialized;
             x2^T k_w^T q_b folds into Q' channel bias)
    F_att   = v_w (x2 A_norm) + v_b  ->  M1 = (proj_w v_w) Z,  Z = x2 E
    gate    = sigmoid(z) = (1 + tanh(z/2)) / 2; the 1/2 folds into the BN
             constants so ACT never leaves the exp/tanh function table.
  Device, per 512-query block j (fusion interleaved INTO the same block,
  trailing exp by `lag` key-tile-pairs; E is a 6-slot ring):
    L[m, ns] = x2^T Q'            (bf16 matmuls, keys m on partitions)
    E = exp(L / 16)               (ACT -> bf16; logits O(1), no max needed)
    Z[c, ns] = sum_m x2t[m,c] E[m, ns]   (bf16, accumulated over 32 m-tiles)
    S[ns] = sum_m E[m, ns]        (bf16 pairwise tree on DVE (2x mode) down to
                                   2 partials, then 2 accumulating
                                   ones[128,128] matmuls -> S broadcast to all
                                   partitions; reciprocal on DVE)
    M1 = P2 Z ; r' = relu(M1*(G/2)*(1/S) + Bc/2)  (DVE STT + ACT relu-bias)
    out = x1 + (1+tanh((gz+gb)/2)) * r'           (DVE STT + Pool add)
  with G = gamma*rsqrt(var+eps), Bc = beta + (proj_b + proj_w v_b - mean)*G.
  Block boundaries interleave the previous block's S/evict/M1/post between
  the next block's logits so the PE never idles (measured ~99.5% PE
  occupancy over the steady state).  All matmul inputs ship as bf16 (x1
  additionally as fp32, streamed late, only for the residual add); x2 ships
  twice (channels-major for logits, keys-major pretransposed for fusion).
  DMA uses 3 DGE rings (SP + ACT + Pool DGE) with transfers >=2KB per
  partition line, per-lane order == first-use order, and small first
  chunks to ride out the per-ring startup ramp.  The final block's post
  chain runs in 256-column chunks split across DVE/Pool lanes.
"""
from contextlib import ExitStack

import numpy as np
import ml_dtypes

import concourse.bass as bass
import concourse.mybir as mybir
import concourse.tile as tile
from concourse import bacc
from concourse.bass_utils import run_bass_kernel_spmd

F32 = mybir.dt.float32
F32R = mybir.dt.float32r
BF16 = mybir.dt.bfloat16
AF = mybir.ActivationFunctionType
OP = mybir.AluOpType

B, C, H, W = 4, 256, 64, 64
N = H * W            # 4096
NCORES = 8
NH = N // 2          # 2048 queries per core
NBLK = 512           # query block
NBLOCKS = NH // NBLK
MT = N // 128        # 32 key tiles
MT2 = MT // 2        # 16 exp steps per block
EPS = 1e-5
SCALE = float(C) ** -0.5


def build():
    nc = bacc.Bacc("TRN2", target_bir_lowering=False, debug=False,
                   num_devices=NCORES)
    x1f_d = nc.dram_tensor("x1f", [C, NH], F32R, kind="ExternalInput")
    x1b_d = nc.dram_tensor("x1b", [C, NH], BF16, kind="ExternalInput")
    x2rb_d = nc.dram_tensor("x2rb", [C, N], BF16, kind="ExternalInput")
    x2tb_d = nc.dram_tensor("x2tb", [128, MT * C], BF16, kind="ExternalInput")
    wqp_d = nc.dram_tensor("wqp", [128, 512], BF16, kind="ExternalInput")
    bb_d = nc.dram_tensor("bb", [128, 1024], BF16, kind="ExternalInput")
    vec_d = nc.dram_tensor("vecs", [C, 4], F32, kind="ExternalInput")
    out_d = nc.dram_tensor("out", [C, NH], F32, kind="ExternalOutput")

    with tile.TileContext(nc) as tc, ExitStack() as ctx:
        pers = ctx.enter_context(tc.tile_pool(name="pers", bufs=1))
        work = ctx.enter_context(tc.tile_pool(name="work", bufs=2))
        psum = ctx.enter_context(tc.tile_pool(name="psum", bufs=1, space="PSUM"))

        # ---- persistent tiles ----
        wqp = pers.tile([128, 512], BF16, tag="wqp", name="wqp")
        bb = pers.tile([128, 1024], BF16, tag="bb", name="bb")
        wq = [wqp[:, ci * 256:(ci + 1) * 256] for ci in range(2)]
        p2 = [bb[:, ci * 256:(ci + 1) * 256] for ci in range(2)]
        gw = [bb[:, 512 + ci * 256: 512 + (ci + 1) * 256] for ci in range(2)]
        vec = [pers.tile([128, 4], F32, tag=f"vec{ci}", name=f"vec{ci}") for ci in range(2)]
        x1f = [pers.tile([128, NH], F32R, tag=f"x1f{ci}", name=f"x1f{ci}") for ci in range(2)]
        x1b = [pers.tile([128, NH], BF16, tag=f"x1b{ci}", name=f"x1b{ci}") for ci in range(2)]
        x2rb = [pers.tile([128, N], BF16, tag=f"x2rb{ci}", name=f"x2rb{ci}") for ci in range(2)]
        x2tb = pers.tile([128, MT * C], BF16, tag="x2tb", name="x2tb")
        Qt = [pers.tile([128, NH], BF16, tag=f"Qt{co}", name=f"Qt{co}") for co in range(2)]
        ones_f = pers.tile([128, 128], F32, tag="ones_f", name="ones_f")
        ones_b = pers.tile([128, 128], BF16, tag="ones_b", name="ones_b")

        c0, c1 = slice(0, 128), slice(128, 256)
        cs2 = [c0, c1]

        # ---------- pre: constants + input streaming (3 DGE rings) ----------
        # sync ring leads with the logits-critical x2rb ch0 pieces; wq rides
        # the ACT lane ahead of x2tb; Q'(1)/gate(0) defer until their data
        # lands.  >=2KB per partition line except the crawl-phase pieces.
        with nc.named_scope("pre"):
            nc.vector.memset(ones_f[:], 1.0)
            nc.vector.tensor_copy(ones_b[:], ones_f[:])
            XB = 1024
            for h in range(2):
                hs = slice(h * 512, (h + 1) * 512)
                nc.sync.dma_start(x2rb[0][:, hs], x2rb_d[c0, hs])
                nc.sync.dma_start(x2rb[1][:, hs], x2rb_d[c1, hs])
            nc.gpsimd.dma_start(vec[0][:], vec_d[c0, :])
            nc.gpsimd.dma_start(vec[1][:], vec_d[c1, :])
            nc.scalar.dma_start(wqp[:, 0:256], wqp_d[:, 0:256])
            nc.scalar.dma_start(wqp[:, 256:512], wqp_d[:, 256:512])
            for h in range(2):
                hs = slice(h * 512, (h + 1) * 512)
                nc.gpsimd.dma_start(x1b[0][:, hs], x1b_d[c0, hs])
                nc.gpsimd.dma_start(x1b[1][:, hs], x1b_d[c1, hs])
            nc.scalar.dma_start(x2tb[:, 0:2 * C], x2tb_d[:, 0:2 * C])
            nc.scalar.dma_start(x2tb[:, 2 * C:4 * C], x2tb_d[:, 2 * C:4 * C])
            nc.gpsimd.dma_start(bb[:], bb_d[:, :])
            for ch in range(1, 4):
                chs = slice(ch * XB, (ch + 1) * XB)
                nc.sync.dma_start(x2rb[0][:, chs], x2rb_d[c0, chs])
                nc.gpsimd.dma_start(x2rb[1][:, chs], x2rb_d[c1, chs])
            for ch in range(1, 6):
                ts = slice(ch * 4 * C, (ch + 1) * 4 * C)
                nc.scalar.dma_start(x2tb[:, ts], x2tb_d[:, ts])
            nc.sync.dma_start(x2tb[:, 6 * 4 * C: 7 * 4 * C],
                              x2tb_d[:, 6 * 4 * C: 7 * 4 * C])
            nc.gpsimd.dma_start(x2tb[:, 7 * 4 * C: 8 * 4 * C],
                                x2tb_d[:, 7 * 4 * C: 8 * 4 * C])
            b1 = slice(XB, 2 * XB)
            nc.sync.dma_start(x1b[0][:, b1], x1b_d[c0, b1])
            nc.gpsimd.dma_start(x1b[1][:, b1], x1b_d[c1, b1])
            for half in range(2):
                hs = slice(half * XB, (half + 1) * XB)
                nc.sync.dma_start(x1f[0][:, hs], x1f_d[c0, hs])
                nc.gpsimd.dma_start(x1f[1][:, hs], x1f_d[c1, hs])

        def emit_qproj(nch):
            ns = slice(nch * NBLK, (nch + 1) * NBLK)
            for co in range(2):
                qp = psum.tile([128, NBLK], F32, tag="acc", name="acc", bufs=2)
                for ci in range(2):
                    nc.tensor.matmul(
                        qp[:], wq[ci][:, co * 128:(co + 1) * 128],
                        x1b[ci][:, ns], start=(ci == 0), stop=(ci == 1))
                nc.scalar.activation(Qt[co][:, ns], qp[:], AF.Identity,
                                     bias=vec[co][:, 0:1])

        def emit_gate(j):
            """Gate logits for block j, partition-broadcast via replicated
            gate-weight lhsT; tanh((z+gb)/2) -> tg [128,NBLK] fp32."""
            ns = slice(j * NBLK, (j + 1) * NBLK)
            gp = psum.tile([128, NBLK], F32, tag="acc", name="gp", bufs=2)
            for ci in range(2):
                nc.tensor.matmul(gp[:], gw[ci][:, 0:128], x1b[ci][:, ns],
                                 start=(ci == 0), stop=False)
            for ci in range(2):
                nc.tensor.matmul(gp[:], gw[ci][:, 128:256], x2rb[ci][:, ns],
                                 start=False, stop=(ci == 1))
            tg = work.tile([128, NBLK], F32, tag="tg", name="tg", bufs=2)
            nc.scalar.activation(tg[:], gp[:], AF.Tanh, scale=0.5,
                                 bias=vec[0][:, 3:4])
            if j == NBLOCKS - 1:
                # tail uses Pool tensor_mul, which needs 1+tanh materialized
                tg1 = work.tile([128, NBLK], F32, tag="tg1", name="tg1",
                                bufs=1)
                nc.vector.tensor_scalar_add(tg1[:], tg[:], 1.0)
                return tg1
            return tg

        def s_finalize(j, sacc2):
            """S (sum over keys) broadcast to all partitions, then 1/S."""
            with nc.named_scope(f"sfin{j}"):
                sb = psum.tile([128, NBLK], F32, tag="acc", name="sb", bufs=2)
                nc.tensor.matmul(sb[:], ones_b[:], sacc2[0][:], start=True,
                                 stop=False)
                nc.tensor.matmul(sb[:], ones_b[:], sacc2[1][:], start=False,
                                 stop=True)
                invs = work.tile([128, NBLK], F32, tag="invs", name="invs",
                                 bufs=2)
                nc.vector.reciprocal_approx_fast(invs[:], sb[:])
            return invs

        def emit_m1(Fs, co):
            mp = psum.tile([128, NBLK], F32, tag="acc", name="acc", bufs=2)
            for ci in range(2):
                nc.tensor.matmul(mp[:], p2[ci][:, co * 128:(co + 1) * 128],
                                 Fs[ci][:], start=(ci == 0), stop=(ci == 1))
            return mp

        def post_co(j, co, mp, invs, tg):
            """Normalize + BN + relu + gate + residual + store for one co."""
            ns = slice(j * NBLK, (j + 1) * NBLK)
            cs = cs2[co]
            with nc.named_scope(f"post{j}_{co}"):
                t1 = work.tile([128, NBLK], F32, tag=f"t1{co}", name="t1")
                nc.vector.scalar_tensor_tensor(
                    t1[:], mp[:], vec[co][:, 1:2], invs[:],
                    op0=OP.mult, op1=OP.mult)
                r = work.tile([128, NBLK], F32, tag=f"r{co}", name="r")
                nc.scalar.activation(r[:], t1[:], AF.Relu,
                                     bias=vec[co][:, 2:3])
                rg = work.tile([128, NBLK], F32, tag=f"rg{co}", name="rg")
                nc.vector.scalar_tensor_tensor(rg[:], tg[:], 1.0, r[:],
                                               op0=OP.add, op1=OP.mult)
                ot = work.tile([128, NBLK], F32, tag=f"ot{co}", name="ot")
                nc.gpsimd.tensor_add(ot[:], rg[:],
                                     x1f[co][:, ns].bitcast(F32))
                nc.sync.dma_start(out_d[cs, ns], ot[:])

        def post_tail(j, mps, invs, tg1):
            """Tail post: 256-col chunks; h0 chunks ride Pool, h1 DVE, so the
            two lanes drain in parallel; all stores issue after the compute
            chains (sync + ACT rings in parallel) so a blocked store issue
            never stalls the ACT queue mid-chain."""
            HB = NBLK // 2
            stores = []
            for h in range(2):
                for co in range(2):
                    mp = mps[co]
                    hs = slice(h * HB, (h + 1) * HB)
                    ns = slice(j * NBLK + h * HB, j * NBLK + (h + 1) * HB)
                    with nc.named_scope(f"post{j}_{co}"):
                        t1 = work.tile([128, HB], F32, tag=f"tt{co}{h}",
                                       name="t1")
                        nc.vector.scalar_tensor_tensor(
                            t1[:], mp[:, hs], vec[co][:, 1:2], invs[:, hs],
                            op0=OP.mult, op1=OP.mult)
                        r = work.tile([128, HB], F32, tag=f"tr{co}{h}",
                                      name="r")
                        nc.scalar.activation(r[:], t1[:], AF.Relu,
                                             bias=vec[co][:, 2:3])
                        rg = work.tile([128, HB], F32, tag=f"tg{co}{h}",
                                       name="rg")
                        ot = work.tile([128, HB], F32, tag=f"to{co}{h}",
                                       name="ot")
                        if h == 0:
                            nc.gpsimd.tensor_mul(rg[:], tg1[:, hs], r[:])
                            nc.gpsimd.tensor_add(ot[:], rg[:],
                                                 x1f[co][:, ns].bitcast(F32))
                        else:
                            nc.vector.tensor_mul(rg[:], tg1[:, hs], r[:])
                            nc.vector.tensor_add(ot[:], rg[:],
                                                 x1f[co][:, ns].bitcast(F32))
                        stores.append((out_d[cs2[co], ns], ot))
            for k, (dst, ot) in enumerate(stores):
                ring = nc.sync if k % 2 == 0 else nc.scalar
                ring.dma_start(dst, ot[:])

        def emit_block(j, boundary, lag=2):
            """Logits+exp+fusion for block j; fusion trails exp by `lag` steps.
            boundary(k) emits interleaved PE work after logits step k."""
            ns = slice(j * NBLK, (j + 1) * NBLK)
            slots = [None] * 6
            sacc2 = []

            def feed(t, lvl):
                if lvl == 5:
                    sacc2.append(t)
                    return
                if slots[lvl] is None:
                    slots[lvl] = t
                    return
                prev = slots[lvl]
                slots[lvl] = None
                nt = work.tile([128, NBLK], BF16, tag=f"tree{lvl}",
                               name=f"tree{lvl}", bufs=2)
                nc.vector.tensor_add(nt[:], prev[:], t[:])
                feed(nt, lvl + 1)

            fp = [psum.tile([128, NBLK], F32, tag=f"F{co}", name=f"F{co}",
                            bufs=1) for co in range(2)]
            Ets = [None] * MT2

            def fusion_step(mt2):
                Et = Ets[mt2]
                for sub in range(2):
                    mt = 2 * mt2 + sub
                    es = slice(sub * NBLK, (sub + 1) * NBLK)
                    for co in range(2):
                        nc.tensor.matmul(
                            fp[co][:],
                            x2tb[:, mt * C + co * 128: mt * C + (co + 1) * 128],
                            Et[:, es], start=(mt == 0), stop=(mt == MT - 1))

            for mt2 in range(MT2):
                lp = psum.tile([128, 2 * NBLK], F32, tag="L", name="L", bufs=2)
                for sub in range(2):
                    mt = 2 * mt2 + sub
                    msl = slice(mt * 128, (mt + 1) * 128)
                    for ci in range(2):
                        nc.tensor.matmul(
                            lp[:, sub * NBLK:(sub + 1) * NBLK],
                            x2rb[ci][:, msl], Qt[ci][:, ns],
                            start=(ci == 0), stop=(ci == 1))
                if boundary is not None:
                    boundary(mt2)
                Et = work.tile([128, 2 * NBLK], BF16, tag="E", name="E",
                               bufs=6)
                nc.scalar.activation(Et[:], lp[:], AF.Exp, scale=SCALE)
                Ets[mt2] = Et
                if mt2 >= lag:
                    fusion_step(mt2 - lag)
                # softmax-sum tree (DVE, bf16 2x): pair within Et, then fold
                p1 = work.tile([128, NBLK], BF16, tag="tree1", name="tree1",
                               bufs=2)
                nc.vector.tensor_add(p1[:], Et[:, 0:NBLK], Et[:, NBLK:2 * NBLK])
                feed(p1, 2)
            for k in range(MT2 - lag, MT2):
                fusion_step(k)
            assert len(sacc2) == 2
            return fp, sacc2

        tg = [None] * NBLOCKS
        invs = [None] * NBLOCKS
        fps = [None] * NBLOCKS
        saccs = [None] * NBLOCKS

        # ---------- block 0: Q'(0) + gate(0) first, Q'(1..3) interleaved ----
        with nc.named_scope("blk0"):
            emit_qproj(0)

            def boundary0(k):
                if k == 10:
                    emit_qproj(1)
                elif k == 12:
                    tg[0] = emit_gate(0)

            fps[0], saccs[0] = emit_block(0, boundary0, lag=4)

        # ---------- blocks 1..3 with previous block's post interleaved ----
        for j in range(1, NBLOCKS):
            p = j - 1

            def boundary(k, p=p, j=j):
                # PE-order interleave: gate(j) early, then S/M1 of block p
                # spaced between logits steps so PE never waits.
                if k == 0:
                    tg[j] = emit_gate(j)
                elif k == 1:
                    invs[p] = s_finalize(p, saccs[p])
                elif k == 2:
                    Fs = [work.tile([128, NBLK], BF16, tag=f"Fs{co}",
                                    name=f"Fs{co}", bufs=2) for co in range(2)]
                    for co in range(2):
                        nc.vector.tensor_copy(Fs[co][:], fps[p][co][:])
                    fps[p] = Fs
                elif k == 3:
                    mp = emit_m1(fps[p], 0)
                    post_co(p, 0, mp, invs[p], tg[p])
                elif k == 4:
                    mp = emit_m1(fps[p], 1)
                    post_co(p, 1, mp, invs[p], tg[p])
                elif k == 5 and j + 1 < NBLOCKS:
                    emit_qproj(j + 1)

            with nc.named_scope(f"blk{j}"):
                fps[j], saccs[j] = emit_block(j, boundary)

        # ---------- tail: block 3 post, chunked + ACT evictions ----------
        p = NBLOCKS - 1
        with nc.named_scope("tail"):
            Fs = [work.tile([128, NBLK], BF16, tag=f"Fs{co}", name=f"Fs{co}",
                            bufs=2) for co in range(2)]
            for co in range(2):
                nc.scalar.activation(Fs[co][:], fps[p][co][:], AF.Copy)
            invs[p] = s_finalize(p, saccs[p])
            mps = [emit_m1(Fs, co) for co in range(2)]
            post_tail(p, mps, invs[p], tg[p])
    nc.compile()
    return nc


_NC = None


def _get_nc():
    global _NC
    if _NC is None:
        _NC = build()
    return _NC


def kernel(**inputs):
    x1 = np.ascontiguousarray(np.asarray(inputs["x1"], dtype=np.float32)).reshape(B, C, N)
    x2 = np.ascontiguousarray(np.asarray(inputs["x2"], dtype=np.float32)).reshape(B, C, N)
    q_w = np.asarray(inputs["q_w"], np.float64)
    k_w = np.asarray(inputs["k_w"], np.float64)
    v_w = np.asarray(inputs["v_w"], np.float64)
    p_w = np.asarray(inputs["proj_w"], np.float64)
    q_b = np.asarray(inputs["q_b"], np.float64)
    v_b = np.asarray(inputs["v_b"], np.float64)
    p_b = np.asarray(inputs["proj_b"], np.float64)
    gamma = np.asarray(inputs["bn_gamma"], np.float64)
    beta = np.asarray(inputs["bn_beta"], np.float64)
    mean = np.asarray(inputs["bn_mean"], np.float64)
    var = np.asarray(inputs["bn_var"], np.float64)
    gate_w = np.asarray(inputs["gate_w"], np.float64)
    gate_b = np.asarray(inputs["gate_b"], np.float64)

    # folded weights: Q' = (k_w^T q_w) x1 + k_w^T q_b ;  M1 = (proj_w v_w) Z
    wq = np.asarray(q_w.T @ k_w, np.float32).astype(ml_dtypes.bfloat16)
    wqp = np.ascontiguousarray(np.concatenate([wq[0:128, :], wq[128:256, :]],
                                              axis=1))
    p2 = np.asarray(v_w.T @ p_w.T, np.float32).astype(ml_dtypes.bfloat16)
    # gate lhsT, replicated along the output-partition dim: [x1 part | x2 part]
    gwrep = np.concatenate([
        np.repeat(gate_w[0, :C].astype(np.float32)[:, None], 128, axis=1),
        np.repeat(gate_w[0, C:].astype(np.float32)[:, None], 128, axis=1),
    ], axis=1).astype(ml_dtypes.bfloat16)
    bb = np.ascontiguousarray(np.concatenate(
        [p2[0:128, :], p2[128:256, :], gwrep[0:128, :], gwrep[128:256, :]],
        axis=1))
    G = gamma / np.sqrt(var + EPS)
    Bc = beta + (p_b + p_w @ v_b - mean) * G
    qpb = k_w.T @ q_b
    gb2 = np.full(C, float(gate_b[0]) * 0.5)
    vecs = np.ascontiguousarray(
        np.stack([qpb, G * 0.5, Bc * 0.5, gb2], axis=1).astype(np.float32))

    in_maps = []
    for core in range(NCORES):
        b, half = divmod(core, 2)
        hq = slice(half * NH, (half + 1) * NH)
        ho = slice((1 - half) * NH, (2 - half) * NH)
        x1q = np.ascontiguousarray(x1[b][:, hq])
        x2p = np.ascontiguousarray(np.concatenate([x2[b][:, hq], x2[b][:, ho]],
                                                  axis=1))
        x2pb = x2p.astype(ml_dtypes.bfloat16)
        # x2 pretransposed into the fusion lhsT SBUF layout:
        # x2tb[p, mt*C + c] = x2p[c, mt*128 + p]
        x2tb = np.ascontiguousarray(
            x2pb.reshape(C, MT, 128).transpose(2, 1, 0).reshape(128, MT * C))
        in_maps.append({
            "x1f": x1q, "x1b": x1q.astype(ml_dtypes.bfloat16),
            "x2rb": np.ascontiguousarray(x2pb), "x2tb": x2tb,
            "wqp": wqp, "bb": bb, "vecs": vecs,
        })

    nc = _get_nc()
    res = run_bass_kernel_spmd(nc, in_maps, core_ids=list(range(NCORES)))
    out = np.empty((B, C, N), np.float32)
    for core in range(NCORES):
        b, half = divmod(core, 2)
        out[b, :, half * NH:(half + 1) * NH] = res.results[core]["out"]
    return out.reshape(B, C, H, W)


# revision 30
# speedup vs baseline: 1.0252x; 1.0252x over previous
"""CrossAttentionFusion Trainium2 kernel.

Full inputs -> shard (batch x query-half) over 8 NeuronCores -> full output.

Per core (batch b = core//2, query half h = core%2, NH=2048 queries):
  Algebraic folding (host precompute):
    L[m,n] = K^T Q = x2^T (k_w^T q_w) x1 =: x2^T Q'   (K never materialized;
             x2^T k_w^T q_b folds into Q' channel bias)
    F_att   = v_w (x2 A_norm) + v_b  ->  M1 = (proj_w v_w) Z,  Z = x2 E
    gate    = sigmoid(z) = (1 + tanh(z/2)) / 2; the 1/2 folds into the BN
             constants so ACT never leaves the exp/tanh function table.
  Device, per 512-query block j (fusion interleaved INTO the same block,
  trailing exp by `lag` key-tile-pairs; E is a 6-slot ring):
    L[m, ns] = x2^T Q'            (bf16 matmuls, keys m on partitions)
    E = exp(L / 16)               (ACT -> bf16; logits O(1), no max needed)
    Z[c, ns] = sum_m x2t[m,c] E[m, ns]   (bf16, accumulated over 32 m-tiles)
    S[ns] = sum_m E[m, ns]        (bf16 pairwise tree on DVE (2x mode) down to
                                   2 partials, then 2 accumulating
                                   ones[128,128] matmuls -> S broadcast to all
                                   partitions; reciprocal on DVE)
    M1 = P2 Z ; r' = relu(M1*(G/2)*(1/S) + Bc/2)  (DVE STT + ACT relu-bias)
    out = x1 + (1+tanh((gz+gb)/2)) * r'           (DVE STT + Pool add)
  with G = gamma*rsqrt(var+eps), Bc = beta + (proj_b + proj_w v_b - mean)*G.
  Block boundaries interleave the previous block's S/evict/M1/post between
  the next block's logits so the PE never idles (measured ~99.5% PE
  occupancy over the steady state).  All matmul inputs ship as bf16 (x1
  additionally as fp32, streamed late, only for the residual add); x2 ships
  twice (channels-major for logits, keys-major pretransposed for fusion).
  DMA uses 3 DGE rings (SP + ACT + Pool DGE) with transfers >=2KB per
  partition line, per-lane order == first-use order, and small first
  chunks to ride out the per-ring startup ramp.  The final block's post
  chain runs in 256-column chunks split across DVE/Pool lanes.
"""
from contextlib import ExitStack

import numpy as np
import ml_dtypes

import concourse.bass as bass
import concourse.mybir as mybir
import concourse.tile as tile
from concourse import bacc
from concourse.bass_utils import run_bass_kernel_spmd

F32 = mybir.dt.float32
F32R = mybir.dt.float32r
BF16 = mybir.dt.bfloat16
AF = mybir.ActivationFunctionType
OP = mybir.AluOpType

B, C, H, W = 4, 256, 64, 64
N = H * W            # 4096
NCORES = 8
NH = N // 2          # 2048 queries per core
NBLK = 512           # query block
NBLOCKS = NH // NBLK
MT = N // 128        # 32 key tiles
MT2 = MT // 2        # 16 exp steps per block
EPS = 1e-5
SCALE = float(C) ** -0.5


def build():
    nc = bacc.Bacc("TRN2", target_bir_lowering=False, debug=False,
                   num_devices=NCORES)
    x1f_d = nc.dram_tensor("x1f", [C, NH], F32R, kind="ExternalInput")
    x1b_d = nc.dram_tensor("x1b", [C, NH], BF16, kind="ExternalInput")
    x2rb_d = nc.dram_tensor("x2rb", [C, N], BF16, kind="ExternalInput")
    x2tb_d = nc.dram_tensor("x2tb", [128, MT * C], BF16, kind="ExternalInput")
    wqp_d = nc.dram_tensor("wqp", [128, 512], BF16, kind="ExternalInput")
    bb_d = nc.dram_tensor("bb", [128, 1024], BF16, kind="ExternalInput")
    vec_d = nc.dram_tensor("vecs", [C, 4], F32, kind="ExternalInput")
    out_d = nc.dram_tensor("out", [C, NH], F32, kind="ExternalOutput")

    with tile.TileContext(nc) as tc, ExitStack() as ctx:
        pers = ctx.enter_context(tc.tile_pool(name="pers", bufs=1))
        work = ctx.enter_context(tc.tile_pool(name="work", bufs=2))
        psum = ctx.enter_context(tc.tile_pool(name="psum", bufs=1, space="PSUM"))

        # ---- persistent tiles ----
        wqp = pers.tile([128, 512], BF16, tag="wqp", name="wqp")
        bb = pers.tile([128, 1024], BF16, tag="bb", name="bb")
        wq = [wqp[:, ci * 256:(ci + 1) * 256] for ci in range(2)]
        p2 = [bb[:, ci * 256:(ci + 1) * 256] for ci in range(2)]
        gw = [bb[:, 512 + ci * 256: 512 + (ci + 1) * 256] for ci in range(2)]
        vec = [pers.tile([128, 4], F32, tag=f"vec{ci}", name=f"vec{ci}") for ci in range(2)]
        x1f = [pers.tile([128, NH], F32R, tag=f"x1f{ci}", name=f"x1f{ci}") for ci in range(2)]
        x1b = [pers.tile([128, NH], BF16, tag=f"x1b{ci}", name=f"x1b{ci}") for ci in range(2)]
        x2rb = [pers.tile([128, N], BF16, tag=f"x2rb{ci}", name=f"x2rb{ci}") for ci in range(2)]
        x2tb = pers.tile([128, MT * C], BF16, tag="x2tb", name="x2tb")
        Qt = [pers.tile([128, NH], BF16, tag=f"Qt{co}", name=f"Qt{co}") for co in range(2)]
        ones_f = pers.tile([128, 128], F32, tag="ones_f", name="ones_f")
        ones_b = pers.tile([128, 128], BF16, tag="ones_b", name="ones_b")

        c0, c1 = slice(0, 128), slice(128, 256)
        cs2 = [c0, c1]

        # ---------- pre: constants + input streaming (3 DGE rings) ----------
        # sync ring leads with the logits-critical x2rb ch0 pieces; wq rides
        # the ACT lane ahead of x2tb; Q'(1)/gate(0) defer until their data
        # lands.  >=2KB per partition line except the crawl-phase pieces.
        with nc.named_scope("pre"):
            nc.vector.memset(ones_f[:], 1.0)
            nc.vector.tensor_copy(ones_b[:], ones_f[:])
            XB = 1024
            for h in range(2):
                hs = slice(h * 512, (h + 1) * 512)
                nc.sync.dma_start(x2rb[0][:, hs], x2rb_d[c0, hs])
                nc.sync.dma_start(x2rb[1][:, hs], x2rb_d[c1, hs])
            nc.gpsimd.dma_start(vec[0][:], vec_d[c0, :])
            nc.gpsimd.dma_start(vec[1][:], vec_d[c1, :])
            nc.scalar.dma_start(wqp[:], wqp_d[:, :])
            for h in range(2):
                hs = slice(h * 512, (h + 1) * 512)
                nc.gpsimd.dma_start(x1b[0][:, hs], x1b_d[c0, hs])
                nc.gpsimd.dma_start(x1b[1][:, hs], x1b_d[c1, hs])
            nc.scalar.dma_start(x2tb[:, 0:2 * C], x2tb_d[:, 0:2 * C])
            nc.scalar.dma_start(x2tb[:, 2 * C:4 * C], x2tb_d[:, 2 * C:4 * C])
            nc.gpsimd.dma_start(bb[:], bb_d[:, :])
            for ch in range(1, 4):
                chs = slice(ch * XB, (ch + 1) * XB)
                nc.sync.dma_start(x2rb[0][:, chs], x2rb_d[c0, chs])
                nc.gpsimd.dma_start(x2rb[1][:, chs], x2rb_d[c1, chs])
            for ch in range(1, 6):
                ts = slice(ch * 4 * C, (ch + 1) * 4 * C)
                nc.scalar.dma_start(x2tb[:, ts], x2tb_d[:, ts])
            nc.sync.dma_start(x2tb[:, 6 * 4 * C: 7 * 4 * C],
                              x2tb_d[:, 6 * 4 * C: 7 * 4 * C])
            nc.gpsimd.dma_start(x2tb[:, 7 * 4 * C: 8 * 4 * C],
                                x2tb_d[:, 7 * 4 * C: 8 * 4 * C])
            b1 = slice(XB, 2 * XB)
            nc.sync.dma_start(x1b[0][:, b1], x1b_d[c0, b1])
            nc.gpsimd.dma_start(x1b[1][:, b1], x1b_d[c1, b1])
            for half in range(2):
                hs = slice(half * XB, (half + 1) * XB)
                nc.sync.dma_start(x1f[0][:, hs], x1f_d[c0, hs])
                nc.gpsimd.dma_start(x1f[1][:, hs], x1f_d[c1, hs])

        def emit_qproj(nch):
            ns = slice(nch * NBLK, (nch + 1) * NBLK)
            for co in range(2):
                qp = psum.tile([128, NBLK], F32, tag="acc", name="acc", bufs=2)
                for ci in range(2):
                    nc.tensor.matmul(
                        qp[:], wq[ci][:, co * 128:(co + 1) * 128],
                        x1b[ci][:, ns], start=(ci == 0), stop=(ci == 1))
                nc.scalar.activation(Qt[co][:, ns], qp[:], AF.Identity,
                                     bias=vec[co][:, 0:1])

        def emit_gate(j):
            """Gate logits for block j, partition-broadcast via replicated
            gate-weight lhsT; tanh((z+gb)/2) -> tg [128,NBLK] fp32."""
            ns = slice(j * NBLK, (j + 1) * NBLK)
            gp = psum.tile([128, NBLK], F32, tag="acc", name="gp", bufs=2)
            for ci in range(2):
                nc.tensor.matmul(gp[:], gw[ci][:, 0:128], x1b[ci][:, ns],
                                 start=(ci == 0), stop=False)
            for ci in range(2):
                nc.tensor.matmul(gp[:], gw[ci][:, 128:256], x2rb[ci][:, ns],
                                 start=False, stop=(ci == 1))
            tg = work.tile([128, NBLK], F32, tag="tg", name="tg", bufs=2)
            nc.scalar.activation(tg[:], gp[:], AF.Tanh, scale=0.5,
                                 bias=vec[0][:, 3:4])
            if j == NBLOCKS - 1:
                # tail uses Pool tensor_mul, which needs 1+tanh materialized
                tg1 = work.tile([128, NBLK], F32, tag="tg1", name="tg1",
                                bufs=1)
                nc.vector.tensor_scalar_add(tg1[:], tg[:], 1.0)
                return tg1
            return tg

        def s_finalize(j, sacc2):
            """S (sum over keys) broadcast to all partitions, then 1/S."""
            with nc.named_scope(f"sfin{j}"):
                sb = psum.tile([128, NBLK], F32, tag="acc", name="sb", bufs=2)
                nc.tensor.matmul(sb[:], ones_b[:], sacc2[0][:], start=True,
                                 stop=False)
                nc.tensor.matmul(sb[:], ones_b[:], sacc2[1][:], start=False,
                                 stop=True)
                invs = work.tile([128, NBLK], F32, tag="invs", name="invs",
                                 bufs=2)
                nc.vector.reciprocal_approx_fast(invs[:], sb[:])
            return invs

        def emit_m1(Fs, co):
            mp = psum.tile([128, NBLK], F32, tag="acc", name="acc", bufs=2)
            for ci in range(2):
                nc.tensor.matmul(mp[:], p2[ci][:, co * 128:(co + 1) * 128],
                                 Fs[ci][:], start=(ci == 0), stop=(ci == 1))
            return mp

        def post_co(j, co, mp, invs, tg):
            """Normalize + BN + relu + gate + residual + store for one co."""
            ns = slice(j * NBLK, (j + 1) * NBLK)
            cs = cs2[co]
            with nc.named_scope(f"post{j}_{co}"):
                t1 = work.tile([128, NBLK], F32, tag=f"t1{co}", name="t1")
                nc.vector.scalar_tensor_tensor(
                    t1[:], mp[:], vec[co][:, 1:2], invs[:],
                    op0=OP.mult, op1=OP.mult)
                r = work.tile([128, NBLK], F32, tag=f"r{co}", name="r")
                nc.scalar.activation(r[:], t1[:], AF.Relu,
                                     bias=vec[co][:, 2:3])
                rg = work.tile([128, NBLK], F32, tag=f"rg{co}", name="rg")
                nc.vector.scalar_tensor_tensor(rg[:], tg[:], 1.0, r[:],
                                               op0=OP.add, op1=OP.mult)
                ot = work.tile([128, NBLK], F32, tag=f"ot{co}", name="ot")
                nc.gpsimd.tensor_add(ot[:], rg[:],
                                     x1f[co][:, ns].bitcast(F32))
                nc.sync.dma_start(out_d[cs, ns], ot[:])

        def post_tail(j, mps, invs, tg1):
            """Tail post: 256-col chunks; h0 chunks ride Pool, h1 DVE, so the
            two lanes drain in parallel; all stores issue after the compute
            chains (sync + ACT rings in parallel) so a blocked store issue
            never stalls the ACT queue mid-chain."""
            HB = NBLK // 2
            stores = []
            for h in range(2):
                for co in range(2):
                    mp = mps[co]
                    hs = slice(h * HB, (h + 1) * HB)
                    ns = slice(j * NBLK + h * HB, j * NBLK + (h + 1) * HB)
                    with nc.named_scope(f"post{j}_{co}"):
                        t1 = work.tile([128, HB], F32, tag=f"tt{co}{h}",
                                       name="t1")
                        nc.vector.scalar_tensor_tensor(
                            t1[:], mp[:, hs], vec[co][:, 1:2], invs[:, hs],
                            op0=OP.mult, op1=OP.mult)
                        r = work.tile([128, HB], F32, tag=f"tr{co}{h}",
                                      name="r")
                        nc.scalar.activation(r[:], t1[:], AF.Relu,
                                             bias=vec[co][:, 2:3])
                        rg = work.tile([128, HB], F32, tag=f"tg{co}{h}",
                                       name="rg")
                        ot = work.tile([128, HB], F32, tag=f"to{co}{h}",
                                       name="ot")
                        if h == 0:
                            nc.gpsimd.tensor_mul(rg[:], tg1[:, hs], r[:])
                            nc.gpsimd.tensor_add(ot[:], rg[:],
                                                 x1f[co][:, ns].bitcast(F32))
                        else:
                            nc.vector.tensor_mul(rg[:], tg1[:, hs], r[:])
                            nc.vector.tensor_add(ot[:], rg[:],
                                                 x1f[co][:, ns].bitcast(F32))
                        stores.append((out_d[cs2[co], ns], ot))
            for k, (dst, ot) in enumerate(stores):
                ring = nc.sync if k % 2 == 0 else nc.scalar
                ring.dma_start(dst, ot[:])

        def emit_block(j, boundary, lag=2):
            """Logits+exp+fusion for block j; fusion trails exp by `lag` steps.
            boundary(k) emits interleaved PE work after logits step k."""
            ns = slice(j * NBLK, (j + 1) * NBLK)
            slots = [None] * 6
            sacc2 = []

            def feed(t, lvl):
                if lvl == 5:
                    sacc2.append(t)
                    return
                if slots[lvl] is None:
                    slots[lvl] = t
                    return
                prev = slots[lvl]
                slots[lvl] = None
                nt = work.tile([128, NBLK], BF16, tag=f"tree{lvl}",
                               name=f"tree{lvl}", bufs=2)
                nc.vector.tensor_add(nt[:], prev[:], t[:])
                feed(nt, lvl + 1)

            fp = [psum.tile([128, NBLK], F32, tag=f"F{co}", name=f"F{co}",
                            bufs=1) for co in range(2)]
            Ets = [None] * MT2

            def fusion_step(mt2):
                Et = Ets[mt2]
                for sub in range(2):
                    mt = 2 * mt2 + sub
                    es = slice(sub * NBLK, (sub + 1) * NBLK)
                    for co in range(2):
                        nc.tensor.matmul(
                            fp[co][:],
                            x2tb[:, mt * C + co * 128: mt * C + (co + 1) * 128],
                            Et[:, es], start=(mt == 0), stop=(mt == MT - 1))

            for mt2 in range(MT2):
                lp = psum.tile([128, 2 * NBLK], F32, tag="L", name="L", bufs=2)
                for sub in range(2):
                    mt = 2 * mt2 + sub
                    msl = slice(mt * 128, (mt + 1) * 128)
                    for ci in range(2):
                        nc.tensor.matmul(
                            lp[:, sub * NBLK:(sub + 1) * NBLK],
                            x2rb[ci][:, msl], Qt[ci][:, ns],
                            start=(ci == 0), stop=(ci == 1))
                if boundary is not None:
                    boundary(mt2)
                Et = work.tile([128, 2 * NBLK], BF16, tag="E", name="E",
                               bufs=6)
                nc.scalar.activation(Et[:], lp[:], AF.Exp, scale=SCALE)
                Ets[mt2] = Et
                if mt2 >= lag:
                    fusion_step(mt2 - lag)
                # softmax-sum tree (DVE, bf16 2x): pair within Et, then fold
                p1 = work.tile([128, NBLK], BF16, tag="tree1", name="tree1",
                               bufs=2)
                nc.vector.tensor_add(p1[:], Et[:, 0:NBLK], Et[:, NBLK:2 * NBLK])
                feed(p1, 2)
            for k in range(MT2 - lag, MT2):
                fusion_step(k)
            assert len(sacc2) == 2
            return fp, sacc2

        tg = [None] * NBLOCKS
        invs = [None] * NBLOCKS
        fps = [None] * NBLOCKS
        saccs = [None] * NBLOCKS

        # ---------- block 0: Q'(0) + gate(0) first, Q'(1..3) interleaved ----
        with nc.named_scope("blk0"):
            emit_qproj(0)

            def boundary0(k):
                if k == 10:
                    emit_qproj(1)
                elif k == 12:
                    tg[0] = emit_gate(0)

            fps[0], saccs[0] = emit_block(0, boundary0, lag=3)

        # ---------- blocks 1..3 with previous block's post interleaved ----
        for j in range(1, NBLOCKS):
            p = j - 1

            def boundary(k, p=p, j=j):
                # PE-order interleave: gate(j) early, then S/M1 of block p
                # spaced between logits steps so PE never waits.
                if k == 0:
                    tg[j] = emit_gate(j)
                elif k == 1:
                    invs[p] = s_finalize(p, saccs[p])
                elif k == 2:
                    Fs = [work.tile([128, NBLK], BF16, tag=f"Fs{co}",
                                    name=f"Fs{co}", bufs=2) for co in range(2)]
                    for co in range(2):
                        nc.vector.tensor_copy(Fs[co][:], fps[p][co][:])
                    fps[p] = Fs
                elif k == 3:
                    mp = emit_m1(fps[p], 0)
                    post_co(p, 0, mp, invs[p], tg[p])
                elif k == 4:
                    mp = emit_m1(fps[p], 1)
                    post_co(p, 1, mp, invs[p], tg[p])
                elif k == 5 and j + 1 < NBLOCKS:
                    emit_qproj(j + 1)

            with nc.named_scope(f"blk{j}"):
                fps[j], saccs[j] = emit_block(j, boundary)

        # ---------- tail: block 3 post, chunked + ACT evictions ----------
        p = NBLOCKS - 1
        with nc.named_scope("tail"):
            Fs = [work.tile([128, NBLK], BF16, tag=f"Fs{co}", name=f"Fs{co}",
                            bufs=2) for co in range(2)]
            for co in range(2):
                nc.scalar.activation(Fs[co][:], fps[p][co][:], AF.Copy)
            invs[p] = s_finalize(p, saccs[p])
            mps = [emit_m1(Fs, co) for co in range(2)]
            post_tail(p, mps, invs[p], tg[p])
    nc.compile()
    return nc


_NC = None


def _get_nc():
    global _NC
    if _NC is None:
        _NC = build()
    return _NC


def kernel(**inputs):
    x1 = np.ascontiguousarray(np.asarray(inputs["x1"], dtype=np.float32)).reshape(B, C, N)
    x2 = np.ascontiguousarray(np.asarray(inputs["x2"], dtype=np.float32)).reshape(B, C, N)
    q_w = np.asarray(inputs["q_w"], np.float64)
    k_w = np.asarray(inputs["k_w"], np.float64)
    v_w = np.asarray(inputs["v_w"], np.float64)
    p_w = np.asarray(inputs["proj_w"], np.float64)
    q_b = np.asarray(inputs["q_b"], np.float64)
    v_b = np.asarray(inputs["v_b"], np.float64)
    p_b = np.asarray(inputs["proj_b"], np.float64)
    gamma = np.asarray(inputs["bn_gamma"], np.float64)
    beta = np.asarray(inputs["bn_beta"], np.float64)
    mean = np.asarray(inputs["bn_mean"], np.float64)
    var = np.asarray(inputs["bn_var"], np.float64)
    gate_w = np.asarray(inputs["gate_w"], np.float64)
    gate_b = np.asarray(inputs["gate_b"], np.float64)

    # folded weights: Q' = (k_w^T q_w) x1 + k_w^T q_b ;  M1 = (proj_w v_w) Z
    wq = np.asarray(q_w.T @ k_w, np.float32).astype(ml_dtypes.bfloat16)
    wqp = np.ascontiguousarray(np.concatenate([wq[0:128, :], wq[128:256, :]],
                                              axis=1))
    p2 = np.asarray(v_w.T @ p_w.T, np.float32).astype(ml_dtypes.bfloat16)
    # gate lhsT, replicated along the output-partition dim: [x1 part | x2 part]
    gwrep = np.concatenate([
        np.repeat(gate_w[0, :C].astype(np.float32)[:, None], 128, axis=1),
        np.repeat(gate_w[0, C:].astype(np.float32)[:, None], 128, axis=1),
    ], axis=1).astype(ml_dtypes.bfloat16)
    bb = np.ascontiguousarray(np.concatenate(
        [p2[0:128, :], p2[128:256, :], gwrep[0:128, :], gwrep[128:256, :]],
        axis=1))
    G = gamma / np.sqrt(var + EPS)
    Bc = beta + (p_b + p_w @ v_b - mean) * G
    qpb = k_w.T @ q_b
    gb2 = np.full(C, float(gate_b[0]) * 0.5)
    vecs = np.ascontiguousarray(
        np.stack([qpb, G * 0.5, Bc * 0.5, gb2], axis=1).astype(np.float32))

    in_maps = []
    for core in range(NCORES):
        b, half = divmod(core, 2)
        hq = slice(half * NH, (half + 1) * NH)
        ho = slice((1 - half) * NH, (2 - half) * NH)
        x1q = np.ascontiguousarray(x1[b][:, hq])
        x2p = np.ascontiguousarray(np.concatenate([x2[b][:, hq], x2[b][:, ho]],
                                                  axis=1))
        x2pb = x2p.astype(ml_dtypes.bfloat16)
        # x2 pretransposed into the fusion lhsT SBUF layout:
        # x2tb[p, mt*C + c] = x2p[c, mt*128 + p]
        x2tb = np.ascontiguousarray(
            x2pb.reshape(C, MT, 128).transpose(2, 1, 0).reshape(128, MT * C))
        in_maps.append({
            "x1f": x1q, "x1b": x1q.astype(ml_dtypes.bfloat16),
            "x2rb": np.ascontiguousarray(x2pb), "x2tb": x2tb,
            "wqp": wqp, "bb": bb, "vecs": vecs,
        })

    nc = _get_nc()
    res = run_bass_kernel_spmd(nc, in_maps, core_ids=list(range(NCORES)))
    out = np.empty((B, C, N), np.float32)
    for core in range(NCORES):
        b, half = divmod(core, 2)
        out[b, :, half * NH:(half + 1) * NH] = res.results[core]["out"]
    return out.reshape(B, C, H, W)
